# revision 40
# baseline (speedup 1.0000x reference)
"""Continuous positional bias kernel for Trainium2 (8 NeuronCores).

Reference computation (per batch b):
    rel[q,k,:] = query_coords[b,q,:] - key_coords[b,k,:]        (2 coords)
    h1 = relu(rel @ w1 + b1)      # (Nq,Nk,128)
    h2 = relu(h1 @ w2 + b2)       # (Nq,Nk,128)
    out[b,:,q,k] = (h2 @ w3 + b3).T  # (heads=8, Nq, Nk)

Layer 1 is linear in rel = q - k, so
    w1^T rel + b1 = (w1^T q + b1) + (-w1^T k) = beta[:,q] + gamma[:,k]
with beta/gamma computed on host.  On device, per query:
    h1 = relu(gamma + beta_col)     DVE tensor_scalar, bf16 src (483ns)
    h2 = relu(w2^T h1 + b2)         PE matmul; relu drain splits DVE/ACT
    out = w3^T h2                   PE matmul, M=8, col-tiled so the 4 mms
                                    of a 2-query pair run concurrently
    copy PSUM->SBUF stage (bf16), large strided DMAs out (sync+gpsimd
    queues), b3 added on host.

The wall is elementwise: h2 must leave PSUM at 1 elem/cycle/lane (one
PSUM read port) and only DVE/ACT reach PSUM, so both run ~89% busy while
the PE (~80%) has slack.  GPSIMD tensor ops measured 14.8us per [128,1024]
tile (generic Q7 ucode) and are unusable; fp8 DoubleRow L3 fails the 2e-2
gate (4.2e-2 measured numerically).  DMA cannot read PSUM.

Sharding: 8 cores x (batch, 256 queries). All weights replicated.
"""

import numpy as np

B, NQ, NK, H, HD = 2, 1024, 1024, 8, 128
NCORES = 8
CPB = NCORES // B          # cores per batch = 4
QPC = NQ // CPB            # queries per core = 256
KT = 512                   # k-tile (matmul moving free dim)
KH = NK // KT              # k halves = 2
NGR = 8                    # pairs per staging round (pair = 2 q = 4 mms)
RQ = 2 * NGR               # queries per staging round = 32

# Engine schedule (16-query cycle). D=vector, A=scalar.
# GPSIMD's generic tensor_scalar ucode measured 14.8us per h1 — unusable.
# Measured per-op: h1 DVE bf16-src 483ns, h2 DVE 1283 / ACT 1126,
# copy [128,512] DVE 658 / ACT ~720.  Balance: DVE = all h1 + 5/16 h2 +
# 3/8 copies; ACT = 11/16 h2 + 5/8 copies.  ps2 stays 3-deep so the PE
# decouples from drain jitter.
H2E = "DAADAADAADAADAAA"           # h2 relu drain per query
CPE = "ADAADADA"                   # stage copy engine per pair

_CACHE = {}


def _build_nc():
    from contextlib import ExitStack

    import concourse.bass as bass
    import concourse.tile as tile
    from concourse import bacc, mybir
    from concourse.alu_op_type import AluOpType

    f32 = mybir.dt.float32
    bf16 = mybir.dt.bfloat16
    Relu = mybir.ActivationFunctionType.Relu

    nc = bacc.Bacc(
        "TRN2",
        target_bir_lowering=False,
        debug=False,
        enable_asserts=True,
        num_devices=NCORES,
    )

    gamma_d = nc.dram_tensor("gamma", (HD, NK), bf16, kind="ExternalInput").ap()
    beta_d = nc.dram_tensor("beta", (HD, QPC), f32, kind="ExternalInput").ap()
    w2_d = nc.dram_tensor("w2", (HD, HD), bf16, kind="ExternalInput").ap()
    w3_d = nc.dram_tensor("w3", (HD, H), bf16, kind="ExternalInput").ap()
    b2_d = nc.dram_tensor("b2", (HD, 1), f32, kind="ExternalInput").ap()
    out_d = nc.dram_tensor("out", (H, QPC, NK), bf16, kind="ExternalOutput").ap()

    with tile.TileContext(nc) as tc:
        with ExitStack() as ctx:
            consts = ctx.enter_context(tc.tile_pool(name="consts", bufs=1))
            h1p = ctx.enter_context(tc.tile_pool(name="h1p", bufs=6))
            h2p = ctx.enter_context(tc.tile_pool(name="h2p", bufs=6))
            stagep = ctx.enter_context(tc.tile_pool(name="stagep", bufs=2))
            ps2 = ctx.enter_context(tc.tile_pool(name="ps2", bufs=3, space="PSUM"))
            ps3 = ctx.enter_context(tc.tile_pool(name="ps3", bufs=2, space="PSUM"))

            # bf16 operands shipped pre-cast from host: 16-bit gamma src
            # engages the DVE packed mode for h1 (483ns vs 748ns f32);
            # bf16 matmul weights avoid the slow f32r paths.  Scalar
            # operands (beta, b2) must stay f32.
            # input DMAs fan out across engine queues: serialized on one
            # queue they cost ~2us each in semaphore overhead (10us fill)
            gamma_b = consts.tile([HD, NK], bf16)
            nc.sync.dma_start(gamma_b, gamma_d)
            beta = consts.tile([HD, QPC], f32)
            nc.gpsimd.dma_start(beta, beta_d)
            w2r = consts.tile([HD, HD], bf16)
            nc.scalar.dma_start(w2r, w2_d)
            w3r = consts.tile([HD, H], bf16)
            nc.gpsimd.dma_start(w3r, w3_d)
            b2 = consts.tile([HD, 1], f32)
            nc.scalar.dma_start(b2, b2_d)

            def make_h1(q):
                h1 = h1p.tile([HD, NK], bf16, tag="h1")
                nc.vector.tensor_scalar(
                    h1, gamma_b, beta[:, q:q + 1], 0.0,
                    AluOpType.add, AluOpType.max,
                )
                return h1

            def drain_h2(q, p2):
                e = H2E[q % 16]
                h2 = h2p.tile([HD, NK], bf16, tag="h2")
                if e == "A":
                    nc.scalar.activation(h2, p2, Relu, bias=b2)
                else:
                    nc.vector.tensor_scalar(
                        h2, p2, b2, 0.0, AluOpType.add, AluOpType.max,
                    )
                return h2

            npairs = QPC // 2
            h1t = {}          # query -> h1 tile (live window)
            h2t = {}          # query -> h2 tile
            p3t = {}          # pair  -> ps3 tile
            stages = {}       # round -> stage tile (bufs=2 double buffer)

            def l2_for(q):
                h1 = h1t.pop(q)
                p2 = ps2.tile([HD, NK], f32, tag="p2")
                for kh in range(KH):
                    nc.tensor.matmul(
                        p2[:, kh * KT:(kh + 1) * KT],
                        w2r,
                        h1[:, kh * KT:(kh + 1) * KT],
                        start=True,
                        stop=True,
                    )
                h2t[q] = drain_h2(q, p2)

            def l3_for(pair):
                # 4 col-tiled M=8 matmuls, emitted back-to-back after both
                # h2 tiles exist so the PE overlaps them (concurrent strips).
                p3 = ps3.tile([128, KT], f32, tag="p3")
                p3t[pair] = p3
                for j in range(4):
                    jq, kh = j // 2, j % 2
                    h2 = h2t[2 * pair + jq]
                    nc.tensor.matmul(
                        p3[32 * j:32 * j + H, :],
                        w3r,
                        h2[:, kh * KT:(kh + 1) * KT],
                        start=True,
                        stop=True,
                        tile_position=(0, 32 * j),
                    )
                h2t.pop(2 * pair)
                h2t.pop(2 * pair + 1)

            def copy_for(pair):
                r, g = pair // NGR, pair % NGR
                if r not in stages:
                    stage = stagep.tile([128, NGR * KT], bf16, tag="stage")
                    stages[r] = stage
                p3 = p3t.pop(pair)
                dst = stages[r][:, g * KT:(g + 1) * KT]
                if CPE[pair % 8] == "A":
                    nc.scalar.copy(dst, p3)
                else:
                    nc.vector.tensor_copy(dst, p3)

            def dma_round(r, half=None):
                # alternate issue queue: sync and gpsimd each carry half the
                # output DMA stream so neither serializes the tail.  For the
                # final round, `half` splits the DMA in two so the first half
                # fires as soon as its copies land.
                q0 = r * RQ
                ngr, g0 = NGR, 0
                if half is not None:
                    ngr = NGR // 2
                    g0 = half * ngr
                    stage = stages[r] if half == 0 else stages.pop(r)
                else:
                    stage = stages.pop(r)
                nrounds = (QPC // 2) // NGR
                for j in range(4):
                    dest = bass.AP(
                        tensor=out_d.tensor,
                        offset=out_d.offset
                        + (q0 + 2 * g0 + (j // 2)) * NK
                        + (j % 2) * KT,
                        ap=[[QPC * NK, H], [2 * NK, ngr], [1, KT]],
                    )
                    # keep gpsimd's SWDGE queue out of the last two rounds so
                    # its ~7us end-of-kernel drain overlaps compute
                    if r >= nrounds - 2:
                        eng = nc.sync
                    elif half is None:
                        eng = nc.sync if (2 * r + j) % 2 == 0 else nc.gpsimd
                    else:
                        eng = nc.gpsimd if half == 0 else nc.sync
                    eng.dma_start(
                        dest, stage[32 * j:32 * j + H, g0 * KT:(g0 + ngr) * KT]
                    )

            # software-pipelined emission: h1 runs 2 queries ahead of L2;
            # L3 runs 2 pairs behind; copies 1 pair behind L3.
            H1_AHEAD = 2
            L3_LAG = 2
            for q in range(H1_AHEAD):
                h1t[q] = make_h1(q)
            for pair in range(npairs + L3_LAG + 1):
                if pair < npairs:
                    for jq in range(2):
                        q = 2 * pair + jq
                        l2_for(q)
                    for jq in range(2):
                        q = 2 * pair + jq
                        if q + H1_AHEAD < QPC:
                            h1t[q + H1_AHEAD] = make_h1(q + H1_AHEAD)
                lp = pair - L3_LAG
                if 0 <= lp < npairs:
                    l3_for(lp)
                cp = pair - L3_LAG - 1
                if 0 <= cp < npairs:
                    copy_for(cp)
                    last_round = cp // NGR == npairs // NGR - 1
                    if last_round and cp % NGR == NGR // 2 - 1:
                        dma_round(cp // NGR, half=0)
                    elif cp % NGR == NGR - 1:
                        if last_round:
                            dma_round(cp // NGR, half=1)
                        else:
                            dma_round(cp // NGR)

    nc.compile()
    return nc


def _get_nc():
    if "nc" not in _CACHE:
        _CACHE["nc"] = _build_nc()
    return _CACHE["nc"]


def make_in_maps(query_coords, key_coords, w1, b1, w2, b2, w3):
    """Host-side shard prep: per-core gamma/beta + replicated weights."""
    qc = np.asarray(query_coords, np.float32)
    kc = np.asarray(key_coords, np.float32)
    w1 = np.asarray(w1, np.float32)
    b1 = np.asarray(b1, np.float32)
    w2 = np.asarray(w2, np.float32)
    b2 = np.asarray(b2, np.float32)
    w3 = np.asarray(w3, np.float32)

    import ml_dtypes

    b2c = np.ascontiguousarray(b2.reshape(HD, 1))
    w2c = np.ascontiguousarray(w2.astype(ml_dtypes.bfloat16))
    w3c = np.ascontiguousarray(w3.astype(ml_dtypes.bfloat16))

    in_maps = []
    for c in range(NCORES):
        b = c // CPB
        q0 = (c % CPB) * QPC
        gamma = np.ascontiguousarray(
            (-(kc[b] @ w1).T).astype(ml_dtypes.bfloat16)         # (128, NK)
        )
        beta = np.ascontiguousarray(
            (qc[b, q0:q0 + QPC] @ w1).T + b1[:, None]            # (128, QPC)
        )
        in_maps.append(
            {"gamma": gamma, "beta": beta, "w2": w2c, "w3": w3c, "b2": b2c}
        )
    return in_maps


def assemble_output(results, b3):
    """Gather per-core [H, QPC, NK] bf16 results into (B, H, NQ, NK) f32."""
    b3 = np.asarray(b3, np.float32)
    out = np.empty((B, H, NQ, NK), np.float32)
    for c in range(NCORES):
        b = c // CPB
        q0 = (c % CPB) * QPC
        out[b, :, q0:q0 + QPC, :] = results[c]["out"].astype(np.float32)
    if np.any(b3):
        out += b3.reshape(1, H, 1, 1)
    return out


def kernel(**inputs):
    from concourse.bass_utils import run_bass_kernel_spmd

    in_maps = make_in_maps(
        inputs["query_coords"],
        inputs["key_coords"],
        inputs["w1"],
        inputs["b1"],
        inputs["w2"],
        inputs["b2"],
        inputs["w3"],
    )
    nc = _get_nc()
    res = run_bass_kernel_spmd(nc, in_maps, list(range(NCORES)))
    return assemble_output(res.results, inputs["b3"])


# revision 41
# speedup vs baseline: 1.0010x; 1.0010x over previous
"""Continuous positional bias kernel for Trainium2 (8 NeuronCores).

Reference computation (per batch b):
    rel[q,k,:] = query_coords[b,q,:] - key_coords[b,k,:]        (2 coords)
    h1 = relu(rel @ w1 + b1)      # (Nq,Nk,128)
    h2 = relu(h1 @ w2 + b2)       # (Nq,Nk,128)
    out[b,:,q,k] = (h2 @ w3 + b3).T  # (heads=8, Nq, Nk)

Layer 1 is linear in rel = q - k, so
    w1^T rel + b1 = (w1^T q + b1) + (-w1^T k) = beta[:,q] + gamma[:,k]
with beta/gamma computed on host.  On device, per query:
    h1 = relu(gamma + beta_col)     DVE tensor_scalar, bf16 src (483ns)
    h2 = relu(w2^T h1 + b2)         PE matmul; relu drain splits DVE/ACT
    out = w3^T h2                   PE matmul, M=8, col-tiled so the 4 mms
                                    of a 2-query pair run concurrently
    copy PSUM->SBUF stage (bf16), large strided DMAs out (sync+gpsimd
    queues), b3 added on host.

The wall is elementwise: h2 must leave PSUM at 1 elem/cycle/lane (one
PSUM read port) and only DVE/ACT reach PSUM, so both run ~89% busy while
the PE (~80%) has slack.  GPSIMD tensor ops measured 14.8us per [128,1024]
tile (generic Q7 ucode) and are unusable; fp8 DoubleRow L3 fails the 2e-2
gate (4.2e-2 measured numerically).  DMA cannot read PSUM.

Sharding: 8 cores x (batch, 256 queries). All weights replicated.
"""

import numpy as np

B, NQ, NK, H, HD = 2, 1024, 1024, 8, 128
NCORES = 8
CPB = NCORES // B          # cores per batch = 4
QPC = NQ // CPB            # queries per core = 256
KT = 512                   # k-tile (matmul moving free dim)
KH = NK // KT              # k halves = 2
NGR = 8                    # pairs per staging round (pair = 2 q = 4 mms)
RQ = 2 * NGR               # queries per staging round = 32

# Engine schedule (16-query cycle). D=vector, A=scalar.
# GPSIMD's generic tensor_scalar ucode measured 14.8us per h1 — unusable.
# Measured per-op: h1 DVE bf16-src 483ns, h2 DVE 1283 / ACT 1126,
# copy [128,512] DVE 658 / ACT ~720.  Balance: DVE = all h1 + 5/16 h2 +
# 3/8 copies; ACT = 11/16 h2 + 5/8 copies.  ps2 stays 3-deep so the PE
# decouples from drain jitter.
H2E = "DAADAADAADAADAAA"           # h2 relu drain per query
CPE = "ADAADADA"                   # stage copy engine per pair

_CACHE = {}


def _build_nc():
    from contextlib import ExitStack

    import concourse.bass as bass
    import concourse.tile as tile
    from concourse import bacc, mybir
    from concourse.alu_op_type import AluOpType

    f32 = mybir.dt.float32
    bf16 = mybir.dt.bfloat16
    Relu = mybir.ActivationFunctionType.Relu

    nc = bacc.Bacc(
        "TRN2",
        target_bir_lowering=False,
        debug=False,
        enable_asserts=True,
        num_devices=NCORES,
    )

    gamma_d = nc.dram_tensor("gamma", (HD, NK), bf16, kind="ExternalInput").ap()
    beta_d = nc.dram_tensor("beta", (HD, QPC), f32, kind="ExternalInput").ap()
    w2_d = nc.dram_tensor("w2", (HD, HD), bf16, kind="ExternalInput").ap()
    w3_d = nc.dram_tensor("w3", (HD, H), bf16, kind="ExternalInput").ap()
    b2_d = nc.dram_tensor("b2", (HD, 1), f32, kind="ExternalInput").ap()
    out_d = nc.dram_tensor("out", (H, QPC, NK), bf16, kind="ExternalOutput").ap()

    with tile.TileContext(nc) as tc:
        with ExitStack() as ctx:
            consts = ctx.enter_context(tc.tile_pool(name="consts", bufs=1))
            h1p = ctx.enter_context(tc.tile_pool(name="h1p", bufs=6))
            h2p = ctx.enter_context(tc.tile_pool(name="h2p", bufs=6))
            stagep = ctx.enter_context(tc.tile_pool(name="stagep", bufs=2))
            ps2 = ctx.enter_context(tc.tile_pool(name="ps2", bufs=3, space="PSUM"))
            ps3 = ctx.enter_context(tc.tile_pool(name="ps3", bufs=2, space="PSUM"))

            # bf16 operands shipped pre-cast from host: 16-bit gamma src
            # engages the DVE packed mode for h1 (483ns vs 748ns f32);
            # bf16 matmul weights avoid the slow f32r paths.  Scalar
            # operands (beta, b2) must stay f32.
            # input DMAs fan out across engine queues: serialized on one
            # queue they cost ~2us each in semaphore overhead (10us fill)
            gamma_b = consts.tile([HD, NK], bf16)
            nc.sync.dma_start(gamma_b, gamma_d)
            beta = consts.tile([HD, QPC], f32)
            nc.gpsimd.dma_start(beta, beta_d)
            w2r = consts.tile([HD, HD], bf16)
            nc.scalar.dma_start(w2r, w2_d)
            w3r = consts.tile([HD, H], bf16)
            nc.gpsimd.dma_start(w3r, w3_d)
            b2 = consts.tile([HD, 1], f32)
            nc.scalar.dma_start(b2, b2_d)

            def make_h1(q):
                h1 = h1p.tile([HD, NK], bf16, tag="h1")
                nc.vector.tensor_scalar(
                    h1, gamma_b, beta[:, q:q + 1], 0.0,
                    AluOpType.add, AluOpType.max,
                )
                return h1

            def drain_h2(q, p2):
                e = H2E[q % 16]
                h2 = h2p.tile([HD, NK], bf16, tag="h2")
                if e == "A":
                    nc.scalar.activation(h2, p2, Relu, bias=b2)
                else:
                    nc.vector.tensor_scalar(
                        h2, p2, b2, 0.0, AluOpType.add, AluOpType.max,
                    )
                return h2

            npairs = QPC // 2
            h1t = {}          # query -> h1 tile (live window)
            h2t = {}          # query -> h2 tile
            p3t = {}          # pair  -> ps3 tile
            stages = {}       # round -> stage tile (bufs=2 double buffer)

            def l2_for(q):
                h1 = h1t.pop(q)
                p2 = ps2.tile([HD, NK], f32, tag="p2")
                for kh in range(KH):
                    nc.tensor.matmul(
                        p2[:, kh * KT:(kh + 1) * KT],
                        w2r,
                        h1[:, kh * KT:(kh + 1) * KT],
                        start=True,
                        stop=True,
                    )
                h2t[q] = drain_h2(q, p2)

            def l3_for(pair):
                # 4 col-tiled M=8 matmuls, emitted back-to-back after both
                # h2 tiles exist so the PE overlaps them (concurrent strips).
                p3 = ps3.tile([128, KT], f32, tag="p3")
                p3t[pair] = p3
                for j in range(4):
                    jq, kh = j // 2, j % 2
                    h2 = h2t[2 * pair + jq]
                    nc.tensor.matmul(
                        p3[32 * j:32 * j + H, :],
                        w3r,
                        h2[:, kh * KT:(kh + 1) * KT],
                        start=True,
                        stop=True,
                        tile_position=(0, 32 * j),
                    )
                h2t.pop(2 * pair)
                h2t.pop(2 * pair + 1)

            def copy_for(pair):
                r, g = pair // NGR, pair % NGR
                if r not in stages:
                    stage = stagep.tile([128, NGR * KT], bf16, tag="stage")
                    stages[r] = stage
                p3 = p3t.pop(pair)
                dst = stages[r][:, g * KT:(g + 1) * KT]
                if CPE[pair % 8] == "A":
                    nc.scalar.copy(dst, p3)
                else:
                    nc.vector.tensor_copy(dst, p3)

            def dma_round(r, half=None):
                # alternate issue queue: sync and gpsimd each carry half the
                # output DMA stream so neither serializes the tail.  For the
                # final round, `half` splits the DMA in two so the first half
                # fires as soon as its copies land.
                q0 = r * RQ
                ngr, g0 = NGR, 0
                if half is not None:
                    ngr = NGR // 2
                    g0 = half * ngr
                    stage = stages[r] if half == 0 else stages.pop(r)
                else:
                    stage = stages.pop(r)
                nrounds = (QPC // 2) // NGR
                for j in range(4):
                    dest = bass.AP(
                        tensor=out_d.tensor,
                        offset=out_d.offset
                        + (q0 + 2 * g0 + (j // 2)) * NK
                        + (j % 2) * KT,
                        ap=[[QPC * NK, H], [2 * NK, ngr], [1, KT]],
                    )
                    # keep gpsimd's SWDGE queue out of the last two rounds so
                    # its ~7us end-of-kernel drain overlaps compute
                    if r >= nrounds - 2:
                        eng = nc.sync
                    elif half is None:
                        eng = nc.sync if (2 * r + j) % 2 == 0 else nc.gpsimd
                    else:
                        eng = nc.gpsimd if half == 0 else nc.sync
                    eng.dma_start(
                        dest, stage[32 * j:32 * j + H, g0 * KT:(g0 + ngr) * KT]
                    )

            # software-pipelined emission: h1 runs 2 queries ahead of L2;
            # L3 runs 2 pairs behind; copies 1 pair behind L3.
            H1_AHEAD = 2
            L3_LAG = 2
            for q in range(H1_AHEAD):
                h1t[q] = make_h1(q)
            for pair in range(npairs + L3_LAG + 1):
                if pair < npairs:
                    for jq in range(2):
                        q = 2 * pair + jq
                        if q + H1_AHEAD < QPC:
                            h1t[q + H1_AHEAD] = make_h1(q + H1_AHEAD)
                        l2_for(q)
                lp = pair - L3_LAG
                if 0 <= lp < npairs:
                    l3_for(lp)
                cp = pair - L3_LAG - 1
                if 0 <= cp < npairs:
                    copy_for(cp)
                    last_round = cp // NGR == npairs // NGR - 1
                    if last_round and cp % NGR == NGR // 2 - 1:
                        dma_round(cp // NGR, half=0)
                    elif cp % NGR == NGR - 1:
                        if last_round:
                            dma_round(cp // NGR, half=1)
                        else:
                            dma_round(cp // NGR)

    nc.compile()
    return nc


def _get_nc():
    if "nc" not in _CACHE:
        _CACHE["nc"] = _build_nc()
    return _CACHE["nc"]


def make_in_maps(query_coords, key_coords, w1, b1, w2, b2, w3):
    """Host-side shard prep: per-core gamma/beta + replicated weights."""
    qc = np.asarray(query_coords, np.float32)
    kc = np.asarray(key_coords, np.float32)
    w1 = np.asarray(w1, np.float32)
    b1 = np.asarray(b1, np.float32)
    w2 = np.asarray(w2, np.float32)
    b2 = np.asarray(b2, np.float32)
    w3 = np.asarray(w3, np.float32)

    import ml_dtypes

    b2c = np.ascontiguousarray(b2.reshape(HD, 1))
    w2c = np.ascontiguousarray(w2.astype(ml_dtypes.bfloat16))
    w3c = np.ascontiguousarray(w3.astype(ml_dtypes.bfloat16))

    in_maps = []
    for c in range(NCORES):
        b = c // CPB
        q0 = (c % CPB) * QPC
        gamma = np.ascontiguousarray(
            (-(kc[b] @ w1).T).astype(ml_dtypes.bfloat16)         # (128, NK)
        )
        beta = np.ascontiguousarray(
            (qc[b, q0:q0 + QPC] @ w1).T + b1[:, None]            # (128, QPC)
        )
        in_maps.append(
            {"gamma": gamma, "beta": beta, "w2": w2c, "w3": w3c, "b2": b2c}
        )
    return in_maps


def assemble_output(results, b3):
    """Gather per-core [H, QPC, NK] bf16 results into (B, H, NQ, NK) f32."""
    b3 = np.asarray(b3, np.float32)
    out = np.empty((B, H, NQ, NK), np.float32)
    for c in range(NCORES):
        b = c // CPB
        q0 = (c % CPB) * QPC
        out[b, :, q0:q0 + QPC, :] = results[c]["out"].astype(np.float32)
    if np.any(b3):
        out += b3.reshape(1, H, 1, 1)
    return out


def kernel(**inputs):
    from concourse.bass_utils import run_bass_kernel_spmd

    in_maps = make_in_maps(
        inputs["query_coords"],
        inputs["key_coords"],
        inputs["w1"],
        inputs["b1"],
        inputs["w2"],
        inputs["b2"],
        inputs["w3"],
    )
    nc = _get_nc()
    res = run_bass_kernel_spmd(nc, in_maps, list(range(NCORES)))
    return assemble_output(res.results, inputs["b3"])


# revision 42
# speedup vs baseline: 1.0078x; 1.0069x over previous
"""Continuous positional bias kernel for Trainium2 (8 NeuronCores).

Reference computation (per batch b):
    rel[q,k,:] = query_coords[b,q,:] - key_coords[b,k,:]        (2 coords)
    h1 = relu(rel @ w1 + b1)      # (Nq,Nk,128)
    h2 = relu(h1 @ w2 + b2)       # (Nq,Nk,128)
    out[b,:,q,k] = (h2 @ w3 + b3).T  # (heads=8, Nq, Nk)

Layer 1 is linear in rel = q - k, so
    w1^T rel + b1 = (w1^T q + b1) + (-w1^T k) = beta[:,q] + gamma[:,k]
with beta/gamma computed on host.  On device, per query:
    h1 = relu(gamma + beta_col)     DVE tensor_scalar, bf16 src (483ns)
    h2 = relu(w2^T h1 + b2)         PE matmul; relu drain splits DVE/ACT
    out = w3^T h2                   PE matmul, M=8, col-tiled so the 4 mms
                                    of a 2-query pair run concurrently
    copy PSUM->SBUF stage (bf16), large strided DMAs out (sync+gpsimd
    queues), b3 added on host.

The wall is elementwise: h2 must leave PSUM at 1 elem/cycle/lane (one
PSUM read port) and only DVE/ACT reach PSUM, so both run ~89% busy while
the PE (~80%) has slack.  GPSIMD tensor ops measured 14.8us per [128,1024]
tile (generic Q7 ucode) and are unusable; fp8 DoubleRow L3 fails the 2e-2
gate (4.2e-2 measured numerically).  DMA cannot read PSUM.

Sharding: 8 cores x (batch, 256 queries). All weights replicated.
"""

import numpy as np

B, NQ, NK, H, HD = 2, 1024, 1024, 8, 128
NCORES = 8
CPB = NCORES // B          # cores per batch = 4
QPC = NQ // CPB            # queries per core = 256
KT = 512                   # k-tile (matmul moving free dim)
KH = NK // KT              # k halves = 2
NGR = 8                    # pairs per staging round (pair = 2 q = 4 mms)
RQ = 2 * NGR               # queries per staging round = 32

# Engine schedule (16-query cycle). D=vector, A=scalar.
# GPSIMD's generic tensor_scalar ucode measured 14.8us per h1 — unusable.
# Measured per-op: h1 DVE bf16-src 483ns, h2 DVE 1283 / ACT 1126,
# copy [128,512] DVE 658 / ACT ~720.  Balance: DVE = all h1 + 5/16 h2 +
# 3/8 copies; ACT = 11/16 h2 + 5/8 copies.  ps2 stays 3-deep so the PE
# decouples from drain jitter.
H2E = "DAADAADAADAADAAA"           # h2 relu drain per query
CPE = "ADAADADA"                   # stage copy engine per pair

_CACHE = {}


def _build_nc():
    from contextlib import ExitStack

    import concourse.bass as bass
    import concourse.tile as tile
    from concourse import bacc, mybir
    from concourse.alu_op_type import AluOpType

    f32 = mybir.dt.float32
    bf16 = mybir.dt.bfloat16
    Relu = mybir.ActivationFunctionType.Relu

    nc = bacc.Bacc(
        "TRN2",
        target_bir_lowering=False,
        debug=False,
        enable_asserts=True,
        num_devices=NCORES,
    )

    gamma_d = nc.dram_tensor("gamma", (HD, NK), bf16, kind="ExternalInput").ap()
    beta_d = nc.dram_tensor("beta", (HD, QPC), f32, kind="ExternalInput").ap()
    w2_d = nc.dram_tensor("w2", (HD, HD), bf16, kind="ExternalInput").ap()
    w3_d = nc.dram_tensor("w3", (HD, H), bf16, kind="ExternalInput").ap()
    b2_d = nc.dram_tensor("b2", (HD, 1), f32, kind="ExternalInput").ap()
    out_d = nc.dram_tensor("out", (H, QPC, NK), bf16, kind="ExternalOutput").ap()

    with tile.TileContext(nc) as tc:
        with ExitStack() as ctx:
            consts = ctx.enter_context(tc.tile_pool(name="consts", bufs=1))
            h1p = ctx.enter_context(tc.tile_pool(name="h1p", bufs=6))
            h2p = ctx.enter_context(tc.tile_pool(name="h2p", bufs=6))
            stagep = ctx.enter_context(tc.tile_pool(name="stagep", bufs=2))
            ps2 = ctx.enter_context(tc.tile_pool(name="ps2", bufs=3, space="PSUM"))
            ps3 = ctx.enter_context(tc.tile_pool(name="ps3", bufs=2, space="PSUM"))

            # bf16 operands shipped pre-cast from host: 16-bit gamma src
            # engages the DVE packed mode for h1 (483ns vs 748ns f32);
            # bf16 matmul weights avoid the slow f32r paths.  Scalar
            # operands (beta, b2) must stay f32.
            # input DMAs fan out across engine queues: serialized on one
            # queue they cost ~2us each in semaphore overhead (10us fill)
            gamma_b = consts.tile([HD, NK], bf16)
            nc.sync.dma_start(gamma_b, gamma_d)
            beta = consts.tile([HD, QPC], f32)
            nc.gpsimd.dma_start(beta, beta_d)
            w2r = consts.tile([HD, HD], bf16)
            nc.scalar.dma_start(w2r, w2_d)
            w3r = consts.tile([HD, H], bf16)
            nc.gpsimd.dma_start(w3r, w3_d)
            b2 = consts.tile([HD, 1], f32)
            nc.scalar.dma_start(b2, b2_d)

            def make_h1(q):
                h1 = h1p.tile([HD, NK], bf16, tag="h1")
                nc.vector.tensor_scalar(
                    h1, gamma_b, beta[:, q:q + 1], 0.0,
                    AluOpType.add, AluOpType.max,
                )
                return h1

            def drain_h2(q, p2):
                e = H2E[q % 16]
                h2 = h2p.tile([HD, NK], bf16, tag="h2")
                if e == "A":
                    nc.scalar.activation(h2, p2, Relu, bias=b2)
                else:
                    nc.vector.tensor_scalar(
                        h2, p2, b2, 0.0, AluOpType.add, AluOpType.max,
                    )
                return h2

            npairs = QPC // 2
            h1t = {}          # query -> h1 tile (live window)
            h2t = {}          # query -> h2 tile
            p3t = {}          # pair  -> ps3 tile
            stages = {}       # round -> stage tile (bufs=2 double buffer)

            def l2_for(q):
                h1 = h1t.pop(q)
                p2 = ps2.tile([HD, NK], f32, tag="p2")
                for kh in range(KH):
                    nc.tensor.matmul(
                        p2[:, kh * KT:(kh + 1) * KT],
                        w2r,
                        h1[:, kh * KT:(kh + 1) * KT],
                        start=True,
                        stop=True,
                    )
                h2t[q] = drain_h2(q, p2)

            def l3_for(pair):
                # 4 col-tiled M=8 matmuls, emitted back-to-back after both
                # h2 tiles exist so the PE overlaps them (concurrent strips).
                p3 = ps3.tile([128, KT], f32, tag="p3")
                p3t[pair] = p3
                for j in range(4):
                    jq, kh = j // 2, j % 2
                    h2 = h2t[2 * pair + jq]
                    nc.tensor.matmul(
                        p3[32 * j:32 * j + H, :],
                        w3r,
                        h2[:, kh * KT:(kh + 1) * KT],
                        start=True,
                        stop=True,
                        tile_position=(0, 32 * j),
                    )
                h2t.pop(2 * pair)
                h2t.pop(2 * pair + 1)

            def copy_for(pair):
                r, g = pair // NGR, pair % NGR
                if r not in stages:
                    stage = stagep.tile([128, NGR * KT], bf16, tag="stage")
                    stages[r] = stage
                p3 = p3t.pop(pair)
                dst = stages[r][:, g * KT:(g + 1) * KT]
                if CPE[pair % 8] == "A":
                    nc.scalar.copy(dst, p3)
                else:
                    nc.vector.tensor_copy(dst, p3)

            def dma_round(r, half=None):
                # alternate issue queue: sync and gpsimd each carry half the
                # output DMA stream so neither serializes the tail.  For the
                # final round, `half` splits the DMA in two so the first half
                # fires as soon as its copies land.
                q0 = r * RQ
                ngr, g0 = NGR, 0
                if half is not None:
                    ngr = NGR // 2
                    g0 = half * ngr
                    stage = stages[r] if half == 0 else stages.pop(r)
                else:
                    stage = stages.pop(r)
                nrounds = (QPC // 2) // NGR
                for j in range(4):
                    dest = bass.AP(
                        tensor=out_d.tensor,
                        offset=out_d.offset
                        + (q0 + 2 * g0 + (j // 2)) * NK
                        + (j % 2) * KT,
                        ap=[[QPC * NK, H], [2 * NK, ngr], [1, KT]],
                    )
                    # keep gpsimd's SWDGE queue out of the last two rounds so
                    # its ~7us end-of-kernel drain overlaps compute
                    if r >= nrounds - 2:
                        eng = nc.sync
                    elif half is None:
                        eng = nc.sync if (2 * r + j) % 2 == 0 else nc.gpsimd
                    else:
                        eng = nc.gpsimd if half == 0 else nc.sync
                    eng.dma_start(
                        dest, stage[32 * j:32 * j + H, g0 * KT:(g0 + ngr) * KT]
                    )

            # software-pipelined emission: h1 runs 2 queries ahead of L2;
            # L3 runs 2 pairs behind; copies 1 pair behind L3.
            H1_AHEAD = 2
            L3_LAG = 2
            for q in range(H1_AHEAD):
                h1t[q] = make_h1(q)
            for pair in range(npairs + L3_LAG + 1):
                # L3 batch first: its h2 inputs are 2 pairs old and nearly
                # always ready, while L2 may wait on drain-gated ps2 slots;
                # FIFO order would otherwise stall ready L3s behind L2.
                lp = pair - L3_LAG
                if 0 <= lp < npairs:
                    l3_for(lp)
                if pair < npairs:
                    for jq in range(2):
                        q = 2 * pair + jq
                        if q + H1_AHEAD < QPC:
                            h1t[q + H1_AHEAD] = make_h1(q + H1_AHEAD)
                        l2_for(q)
                cp = pair - L3_LAG - 1
                if 0 <= cp < npairs:
                    copy_for(cp)
                    last_round = cp // NGR == npairs // NGR - 1
                    if last_round and cp % NGR == NGR // 2 - 1:
                        dma_round(cp // NGR, half=0)
                    elif cp % NGR == NGR - 1:
                        if last_round:
                            dma_round(cp // NGR, half=1)
                        else:
                            dma_round(cp // NGR)

    nc.compile()
    return nc


def _get_nc():
    if "nc" not in _CACHE:
        _CACHE["nc"] = _build_nc()
    return _CACHE["nc"]


def make_in_maps(query_coords, key_coords, w1, b1, w2, b2, w3):
    """Host-side shard prep: per-core gamma/beta + replicated weights."""
    qc = np.asarray(query_coords, np.float32)
    kc = np.asarray(key_coords, np.float32)
    w1 = np.asarray(w1, np.float32)
    b1 = np.asarray(b1, np.float32)
    w2 = np.asarray(w2, np.float32)
    b2 = np.asarray(b2, np.float32)
    w3 = np.asarray(w3, np.float32)

    import ml_dtypes

    b2c = np.ascontiguousarray(b2.reshape(HD, 1))
    w2c = np.ascontiguousarray(w2.astype(ml_dtypes.bfloat16))
    w3c = np.ascontiguousarray(w3.astype(ml_dtypes.bfloat16))

    in_maps = []
    for c in range(NCORES):
        b = c // CPB
        q0 = (c % CPB) * QPC
        gamma = np.ascontiguousarray(
            (-(kc[b] @ w1).T).astype(ml_dtypes.bfloat16)         # (128, NK)
        )
        beta = np.ascontiguousarray(
            (qc[b, q0:q0 + QPC] @ w1).T + b1[:, None]            # (128, QPC)
        )
        in_maps.append(
            {"gamma": gamma, "beta": beta, "w2": w2c, "w3": w3c, "b2": b2c}
        )
    return in_maps


def assemble_output(results, b3):
    """Gather per-core [H, QPC, NK] bf16 results into (B, H, NQ, NK) f32."""
    b3 = np.asarray(b3, np.float32)
    out = np.empty((B, H, NQ, NK), np.float32)
    for c in range(NCORES):
        b = c // CPB
        q0 = (c % CPB) * QPC
        out[b, :, q0:q0 + QPC, :] = results[c]["out"].astype(np.float32)
    if np.any(b3):
        out += b3.reshape(1, H, 1, 1)
    return out


def kernel(**inputs):
    from concourse.bass_utils import run_bass_kernel_spmd

    in_maps = make_in_maps(
        inputs["query_coords"],
        inputs["key_coords"],
        inputs["w1"],
        inputs["b1"],
        inputs["w2"],
        inputs["b2"],
        inputs["w3"],
    )
    nc = _get_nc()
    res = run_bass_kernel_spmd(nc, in_maps, list(range(NCORES)))
    return assemble_output(res.results, inputs["b3"])
